# revision 23
# baseline (speedup 1.0000x reference)
"""Dense-CRF mean-field inference on 8 Trainium2 NeuronCores.

Math restructuring (validated numerically against the jax reference):
  - Kb and Kg share the spatial sigma (5.0), so
        K = Kb + Kg = Kg * (1 + Cc),
    where Cc = exp(-.5||ci-cj||^2/sig_c^2) is a pure COLOR Gaussian.
    Only Cc is input-dependent; Kg (and the x3 UPDATE_FACTOR fold) is
    separable spatial structure the host precomputes as per-block
    rank-1 factors gy[128,14] (x) gx[128,96].
  - Color feature products are <= ~6 in magnitude -> the Cc feature
    matmul is fp16-safe; the whole K band lives in SBUF as fp16
    (1 PE cycle/row vs 4 for fp32). Simulated end-to-end rel err 5e-4
    vs the 2e-2 gate.
  - The Potts 3x3 conv update reduces to out = softmax(input +
    boxsum3(comb)) (class-independent part drops in softmax).
  - Band: 37 global 128-px blocks per core (sim: 37 -> 5e-4, 33 ->
    2.6e-2, so 37 is the minimum safe width). Per-core band order is
    [own 9 | left 14 | right 14] so runtime ds() offsets can split the
    flat copy; out-of-image blocks get gy=0 -> K=0.
  - One fp16 AllGather of the per-core probabilities per iteration.

Sharding: core r owns output image rows [12r, 12r+12).
"""

import os
import sys

import numpy as np

for _p in ("/opt/trn_rl_repo",):
    if _p not in sys.path and os.path.isdir(_p):
        sys.path.insert(0, _p)

H = 96
W = 96
C = 5
N = H * W                      # 9216
NCORES = 8
RPC = H // NCORES              # 12 image rows per core
EXT = RPC + 2                  # 14 rows incl. 1 halo row each side
NLOC = EXT * W                 # 1344 extended-output pixels
NMID = RPC * W                 # 1152 owned pixels
BLK = 128
NBLK = 37                      # K band m-blocks per core
HB = (NBLK - 9) // 2           # 14 blocks each side of the 9 own
GBLK = N // BLK                # 72 global blocks
PADBLK = HB                    # padding blocks each side of flat_pad
FPW = (GBLK + 2 * PADBLK) * C  # flat_pad free width = 500
# matvec n-chunks, row-aligned so the x-box can read PSUM directly
CHROWS = (5, 5, 4)
CHS = [r * W for r in CHROWS]  # (480, 480, 384)
CH0 = [sum(CHS[:j]) for j in range(3)]
BCH = 448                      # build n-chunk (fits one PSUM bank)
# band-local near-block positions (within +-11 blocks of the own window;
# sim: J=11 -> 1.6e-3 rel err, J=9 -> 2e-2). Far blocks use the
# constant-color-factor approximation (1 + mean Cc) * Kg, shipped direct.
NEAR = list(range(0, 9)) + list(range(12, 34))
NFAR = [i for i in range(NBLK) if i not in NEAR]
ITERS = 5

_CACHED_NC = None


def _build_module():
    import concourse.bass as bass
    import concourse.bacc as bacc
    import concourse.tile as tile
    from concourse import mybir
    from concourse.masks import make_identity

    f32 = mybir.dt.float32
    f16 = mybir.dt.float16
    u32 = mybir.dt.uint32
    EXP = mybir.ActivationFunctionType.Exp
    COPY = mybir.ActivationFunctionType.Copy
    ADD = mybir.AluOpType.add
    MULT = mybir.AluOpType.mult

    nc = bacc.Bacc("TRN2", target_bir_lowering=False, debug=False,
                   num_devices=NCORES)

    g_dram = nc.dram_tensor("g_feats", [C, len(NEAR) * BLK], f16,
                            kind="ExternalInput")
    h_dram = nc.dram_tensor("h_feats", [C, NLOC], f16, kind="ExternalInput")
    k16_dram = nc.dram_tensor("k16_init", [BLK, NBLK * NLOC], f16,
                              kind="ExternalInput")
    ipp_dram = nc.dram_tensor("inp_pp", [BLK, GBLK * C], f32, kind="ExternalInput")
    icn_dram = nc.dram_tensor("inp_cn", [C, NMID], f32, kind="ExternalInput")
    off_dram = nc.dram_tensor("offsets", [1, 3], u32, kind="ExternalInput")
    out_dram = nc.dram_tensor("out_loc", [BLK, (NMID // BLK) * C], f32,
                              kind="ExternalOutput")

    def bcast_inner(ap, n):
        return bass.AP(tensor=ap.tensor, offset=ap.offset, ap=[*ap.ap, [0, n]])

    def bcast_mid(ap, n):
        # [p, q] -> [p, n, q] with stride-0 middle dim
        return bass.AP(tensor=ap.tensor, offset=ap.offset,
                       ap=[ap.ap[0], [0, n], *ap.ap[1:]])

    with tile.TileContext(nc) as tc:
        with tc.tile_pool(name="singles", bufs=1) as singles, \
             tc.tile_pool(name="bpsum", bufs=2, space="PSUM") as bppool, \
             tc.tile_pool(name="ipsum", bufs=2, space="PSUM") as ippool, \
             tc.tile_pool(name="iter", bufs=1) as wpool, \
             tc.tile_pool(name="band", bufs=2) as bpool, \
             tc.tile_pool(name="smx", bufs=2) as spool, \
             tc.tile_pool(name="dram", bufs=1, space="DRAM") as dram:

            # ---- long-lived SBUF state ----
            k16 = singles.tile([BLK, NBLK, NLOC], f16, name="k16")
            flat_pad = singles.tile([BLK, FPW], f16, name="flat_pad")
            g_sb = singles.tile([C, len(NEAR) * BLK], f16, name="g_sb")
            h_sb = singles.tile([C, NLOC], f16, name="h_sb")
            ipp_sb = singles.tile([BLK, GBLK * C], f32, name="ipp_sb")
            icn_sb = singles.tile([C, NMID], f32, name="icn_sb")
            ident = singles.tile([BLK, BLK], f32, name="ident")
            off_sb = singles.tile([1, 3], u32, name="off_sb")

            nc.sync.dma_start(out=ipp_sb, in_=ipp_dram[:, :])
            nc.sync.dma_start(out=icn_sb, in_=icn_dram[:, :])
            nc.sync.dma_start(out=off_sb, in_=off_dram[:, :])
            nc.sync.dma_start(out=g_sb, in_=g_dram[:, :])
            nc.sync.dma_start(out=h_sb, in_=h_dram[:, :])
            # k16 initial values (spatial gaussian factors): per near block so
            # the build pipeline starts as soon as each block lands; far
            # blocks (2 contiguous runs) need no device work at all
            for i in NEAR:
                nc.sync.dma_start(
                    out=k16[:, i, :],
                    in_=k16_dram[:, i * NLOC:(i + 1) * NLOC])
            nc.sync.dma_start(
                out=k16[:, NFAR[0]:NFAR[2] + 1, :],
                in_=k16_dram[:, NFAR[0] * NLOC:(NFAR[2] + 1) * NLOC])
            nc.sync.dma_start(
                out=k16[:, NFAR[3]:NFAR[5] + 1, :],
                in_=k16_dram[:, NFAR[3] * NLOC:(NFAR[5] + 1) * NLOC])
            make_identity(nc, ident)
            nc.vector.memset(flat_pad, 0.0)

            # runtime flat_pad element offsets: own / left / right windows
            offs = []
            for j, mx in enumerate(((PADBLK + 9 * (NCORES - 1)) * C,
                                    (PADBLK + 9 * (NCORES - 1) - HB) * C,
                                    (PADBLK + 9 * (NCORES - 1) + 9) * C)):
                regs = nc.alloc_registers(f"off_regs{j}",
                                          engines=(mybir.EngineType.DVE,))
                nc.regs_load(regs, off_sb[0:1, j:j + 1])
                offs.append(nc.snap(regs, donate=True, min_val=0, max_val=mx))
            own_off, left_off, right_off = offs

            # HAM warm-keeper: fp16 matmuls (~213 ns each) that fill PE-idle
            # windows so the activity monitor keeps the PE clock at 2.4 GHz.
            # Each warm's lhsT reads an anchor tile produced just before the
            # idle window -- without the data dependency the static scheduler
            # hoists dep-free matmuls to the very start of the Tensor queue.
            def warm(n, anchor):
                wp = bppool.tile([BLK, 512], f32, tag="pb")
                for _ in range(n):
                    nc.tensor.matmul(wp[0:1, :], anchor[:, 0:1],
                                     k16[:, 0, 0:512], start=True, stop=True)

            def warm32(n, anchor):
                # fp32 anchor with few partitions: 128 fp32 cols = 512 cycles
                wp = bppool.tile([BLK, 512], f32, tag="pb")
                p = anchor.partition_size()
                for _ in range(n):
                    nc.tensor.matmul(wp[0:1, 0:BLK], anchor[:, 0:1],
                                     ident[0:p, 0:BLK], start=True, stop=True)

            ag_in = dram.tile([BLK, (NMID // BLK) * C], f16, name="ag_in")
            ag_out = dram.tile([BLK * NCORES, (NMID // BLK) * C], f16,
                               name="ag_out")
            cc_wu_in = dram.tile([BLK, 1], f16, name="cc_wu_in")
            cc_wu_out = dram.tile([BLK * NCORES, 1], f16, name="cc_wu_out")
            # warm up the collective rings during the build phase; the first
            # AllGather otherwise pays ~35us of one-time setup on the
            # critical path
            nc.gpsimd.collective_compute(
                "AllGather",
                mybir.AluOpType.bypass,
                replica_groups=[list(range(NCORES))],
                ins=[cc_wu_in.opt()],
                outs=[cc_wu_out.opt()],
            )

            # ---- helpers ----
            def softmax_pp(pool, u_pp, mb, tag, out_dt=f32):
                """u_pp: [128, mb*C] logits, pixel-partition layout -> probs."""
                v = u_pp.rearrange("p (a c) -> p a c", c=C)
                mx = pool.tile([BLK, mb], f32, tag=f"{tag}_mx")
                nc.vector.tensor_reduce(out=mx, in_=v,
                                        axis=mybir.AxisListType.X,
                                        op=mybir.AluOpType.max)
                e = pool.tile([BLK, mb * C], f32, tag=f"{tag}_e")
                ev = e.rearrange("p (a c) -> p a c", c=C)
                nc.vector.tensor_sub(ev, v, bcast_inner(mx, C))
                nc.scalar.activation(out=e, in_=e, func=EXP)
                s = pool.tile([BLK, mb], f32, tag=f"{tag}_s")
                nc.vector.tensor_reduce(out=s, in_=ev,
                                        axis=mybir.AxisListType.X,
                                        op=mybir.AluOpType.add)
                nc.vector.reciprocal(out=s, in_=s)
                fl = pool.tile([BLK, mb * C], out_dt, tag=f"{tag}_fl")
                nc.vector.tensor_mul(fl.rearrange("p (a c) -> p a c", c=C), ev,
                                     bcast_inner(s, C))
                return fl

            # ---- phase 2: initial flat = softmax(input), all cores alike ----
            with tc.tile_pool(name="init", bufs=1) as ipool:
                fl0 = softmax_pp(ipool, ipp_sb, GBLK, "sm0", out_dt=f16)
                nc.vector.tensor_copy(
                    out=flat_pad[:, PADBLK * C:(PADBLK + GBLK) * C], in_=fl0)

            # ---- phase 1: build the fp16 K band, fused with iteration
            # 0's matvec (block i's matmuls run right after k16[i] is
            # assembled, hiding iter-0's PE work under the vector-bound
            # build pipeline) ----
            band0 = bpool.tile([BLK, NBLK * C], f16, tag="band")
            nc.vector.tensor_copy(
                out=band0[:, 0:9 * C],
                in_=flat_pad[:, bass.ds(own_off, 9 * C)])
            nc.vector.tensor_copy(
                out=band0[:, 9 * C:(9 + HB) * C],
                in_=flat_pad[:, bass.ds(left_off, HB * C)])
            nc.vector.tensor_copy(
                out=band0[:, (9 + HB) * C:NBLK * C],
                in_=flat_pad[:, bass.ds(right_off, HB * C)])
            pv0 = ippool.tile([BLK, 3, 512], f32, tag="pv")

            def emit0(*blocks):
                blks = list(blocks)
                if 8 in blks:
                    blks += [9, 10, 11]
                for ib in blks:
                    for nb in range(3):
                        nc.tensor.matmul(
                            pv0[0:C, nb, 0:CHS[nb]],
                            band0[:, ib * C:(ib + 1) * C],
                            k16[:, ib, CH0[nb]:CH0[nb] + CHS[nb]],
                            start=(ib == 0), stop=(ib == NBLK - 1))
            with tc.tile_pool(name="kgstage", bufs=3) as kgpool:
                for j, i in enumerate(NEAR):
                    gt = g_sb[:, j * BLK:(j + 1) * BLK]
                    texp = kgpool.tile([BLK, NLOC], f16, tag="texp")
                    for nb in range(3):
                        pb = bppool.tile([BLK, 512], f32, tag="pb")
                        hs = h_sb[:, nb * BCH:(nb + 1) * BCH]
                        nc.tensor.matmul(pb[:, 0:BCH], gt, hs,
                                         start=True, stop=True)
                        nc.scalar.activation(
                            out=texp[:, nb * BCH:(nb + 1) * BCH],
                            in_=pb[:, 0:BCH], func=EXP)
                    ksl = k16[:, i, :]
                    nc.vector.scalar_tensor_tensor(
                        out=ksl, in0=texp, scalar=1.0, in1=ksl,
                        op0=ADD, op1=MULT)
                    # iter-0 matvec contribution, delayed by 2 build steps
                    # so its k16 dependency is settled and the PE FIFO head
                    # never blocks the next block's build matmuls
                    if j >= 2:
                        emit0(NEAR[j - 2])
                emit0(NEAR[-2])
                emit0(NEAR[-1])
                emit0(34, 35, 36)

            # second ring warm-up doubling as a cross-core barrier: absorbs
            # build-phase skew while iteration 0's matvec runs
            nc.gpsimd.collective_compute(
                "AllGather",
                mybir.AluOpType.bypass,
                replica_groups=[list(range(NCORES))],
                ins=[cc_wu_in.opt()],
                outs=[cc_wu_out.opt()],
            )

            # ---- phase 3: iterations ----
            pv_next = None
            band = band0
            pv = None
            for it in range(ITERS):
                # matvec: comb[c, n] = sum_m K[m, n] * flat[c, m]
                # iteration 0 was fused into the build; for later
                # iterations the own 9 blocks were accumulated into
                # pv_next while the AllGather was in flight
                if it == 0:
                    pv = pv0
                else:
                    pv = pv_next
                    band = bpool.tile([BLK, NBLK * C], f16, tag="band")
                    nc.sync.dma_start(
                        out=flat_pad[:, PADBLK * C:(PADBLK + GBLK) * C]
                        .rearrange("p (r j) -> p r j", r=NCORES),
                        in_=ag_out.rearrange("(r p) j -> p r j", p=BLK))
                    nc.vector.tensor_copy(
                        out=band[:, 9 * C:(9 + HB) * C],
                        in_=flat_pad[:, bass.ds(left_off, HB * C)])
                    nc.vector.tensor_copy(
                        out=band[:, (9 + HB) * C:NBLK * C],
                        in_=flat_pad[:, bass.ds(right_off, HB * C)])
                    for nb in range(3):
                        for i in range(9, NBLK):
                            nc.tensor.matmul(
                                pv[0:C, nb, 0:CHS[nb]],
                                band[:, i * C:(i + 1) * C],
                                k16[:, i, CH0[nb]:CH0[nb] + CHS[nb]],
                                start=False, stop=(i == NBLK - 1))

                # 3x3 box sum, x-pass per row-aligned PSUM chunk
                # (hardware: at most one PSUM operand per vector op, so
                # seed t1 with a scalar-engine copy then accumulate);
                # y-pass and transposes are chunk-pipelined so only
                # chunk 2's tail is serial after the matvec. The
                # transposed logits land in chunk 0's already-consumed
                # PSUM bank (pv[:, 0, 0:45]).
                t1 = wpool.tile([C, NLOC], f32, tag="t1")
                t1r = t1.rearrange("p (row x) -> p row x", x=W)
                u = wpool.tile([C, NMID], f32, tag="u")
                ptv = pv[:, 0, 0:45]
                r0 = 0
                for nb in range(3):
                    nr = CHROWS[nb]
                    cb = pv[0:C, nb, 0:CHS[nb]].rearrange(
                        "p (row x) -> p row x", x=W)
                    tb = t1r[:, r0:r0 + nr, :]
                    nc.scalar.activation(out=tb, in_=cb, func=COPY)
                    nc.vector.tensor_add(tb[:, :, 1:W - 1],
                                         tb[:, :, 1:W - 1],
                                         cb[:, :, 0:W - 2])
                    nc.vector.tensor_add(tb[:, :, 1:W - 1],
                                         tb[:, :, 1:W - 1],
                                         cb[:, :, 2:W])
                    nc.vector.tensor_add(tb[:, :, 0:1], tb[:, :, 0:1],
                                         cb[:, :, 0:1])
                    nc.vector.tensor_add(tb[:, :, 0:1], tb[:, :, 0:1],
                                         cb[:, :, 1:2])
                    nc.vector.tensor_add(tb[:, :, W - 1:W],
                                         tb[:, :, W - 1:W],
                                         cb[:, :, W - 1:W])
                    nc.vector.tensor_add(tb[:, :, W - 1:W],
                                         tb[:, :, W - 1:W],
                                         cb[:, :, W - 2:W - 1])
                    r0 += nr
                    if nb == 0:
                        continue
                    if nb == 1:
                        # u rows 0-7 (needs t1 rows 0-9) + transposes 0-5
                        a, b = 0, 8 * W
                    else:
                        # u rows 8-11 (needs t1 rows 8-13) + transposes 6-8
                        a, b = 8 * W, NMID
                    nc.vector.tensor_add(u[:, a:b], t1[:, a:b],
                                         t1[:, a + 2 * W:b + 2 * W])
                    nc.vector.tensor_add(u[:, a:b], u[:, a:b],
                                         t1[:, a + W:b + W])
                    nc.vector.tensor_add(u[:, a:b], u[:, a:b],
                                         icn_sb[:, a:b])
                    for kb in range(a // BLK, b // BLK):
                        nc.tensor.transpose(ptv[:, kb * C:(kb + 1) * C],
                                            u[:, kb * BLK:(kb + 1) * BLK],
                                            ident[0:C, 0:C])
                    warm32(4, u)

                if it < ITERS - 1:
                    flat_l = softmax_pp(spool, ptv, NMID // BLK, "smx",
                                        out_dt=f16)
                    nc.sync.dma_start(out=ag_in, in_=flat_l)
                    nc.gpsimd.collective_compute(
                        "AllGather",
                        mybir.AluOpType.bypass,
                        replica_groups=[list(range(NCORES))],
                        ins=[ag_in.opt()],
                        outs=[ag_out.opt()],
                    )
                    # overlap the AllGather with next iteration's own-block
                    # matmul accumulation (flat_l is this core's own data)
                    pv_next = ippool.tile([BLK, 3, 512], f32, tag="pv")
                    for nb in range(3):
                        for j in range(9):
                            nc.tensor.matmul(
                                pv_next[0:C, nb, 0:CHS[nb]],
                                flat_l[:, j * C:(j + 1) * C],
                                k16[:, j, CH0[nb]:CH0[nb] + CHS[nb]],
                                start=(j == 0), stop=False)
                    warm(34, flat_l)
                else:
                    flat_l = softmax_pp(spool, ptv, NMID // BLK, "smx",
                                        out_dt=f32)
                    nc.sync.dma_start(out=out_dram[:, :], in_=flat_l)

    nc.compile()
    return nc


def _host_inputs(input_tensor, reference_tensor):
    logits = np.ascontiguousarray(
        np.asarray(input_tensor, dtype=np.float32)[0].reshape(C, N))
    ref = np.asarray(reference_tensor, dtype=np.float32)[0]  # [3, 96, 96]

    RGB = (ref / 0.5).reshape(3, N).astype(np.float32)
    c2 = (-0.5 * (RGB * RGB).sum(axis=0)).astype(np.float32)
    ones = np.ones(N, np.float32)
    G_all = np.stack([RGB[0], RGB[1], RGB[2], c2, ones]).astype(np.float16)
    H_all = np.stack([RGB[0], RGB[1], RGB[2], ones, c2]).astype(np.float16)

    # mean color-kernel value for the far-block constant approximation
    samp = RGB[:, ::37]
    d2 = ((samp[:, :, None] - samp[:, None, :]) ** 2).sum(axis=0)
    tbar = float(np.exp(-0.5 * d2).mean())

    # input in pixel-partition layout [128, 72*5]
    ipp = np.ascontiguousarray(
        logits.reshape(C, GBLK, BLK).transpose(2, 1, 0).reshape(BLK, GBLK * C))

    # spatial gaussian tables; x table carries the 3.0 UPDATE_FACTOR fold
    dtab = np.exp(-(np.arange(-(H - 1), H) ** 2) / 50.0)
    gy1 = dtab.astype(np.float32)
    gx3 = (3.0 * dtab).astype(np.float32)
    yy_all = (np.arange(N) // W).astype(np.int64)
    xx_all = (np.arange(N) % W).astype(np.int64)

    def k16_for_core(r, order, yext):
        k = np.zeros((BLK, NBLK, NLOC), np.float16)
        xn = np.arange(W)
        for i, gb in enumerate(order):
            if 0 <= gb < GBLK:
                pm = np.arange(gb * BLK, (gb + 1) * BLK)
                A = gy1[yy_all[pm][:, None] - yext[None, :] + H - 1]
                B = gx3[xx_all[pm][:, None] - xn[None, :] + H - 1]
                kg = (A[:, :, None] * B[:, None, :]).reshape(BLK, NLOC)
                if i in NFAR:
                    kg *= (1.0 + tbar)
                k[:, i, :] = kg.astype(np.float16)
        return k.reshape(BLK, NBLK * NLOC)

    in_maps = []
    k16_interior = None
    for r in range(NCORES):
        # band-local order: [own 9 | left 14 | right 14] global blocks
        order = (list(range(9 * r, 9 * r + 9))
                 + list(range(9 * r - HB, 9 * r))
                 + list(range(9 * r + 9, 9 * r + 9 + HB)))
        yext = np.clip(np.arange(RPC * r - 1, RPC * (r + 1) + 1), 0, H - 1)
        g = np.zeros((C, len(NEAR) * BLK), np.float16)
        for j, i in enumerate(NEAR):
            gb = order[i]
            if 0 <= gb < GBLK:
                g[:, j * BLK:(j + 1) * BLK] = G_all[:, gb * BLK:(gb + 1) * BLK]
        if 2 <= r <= 5:
            if k16_interior is None:
                k16_interior = k16_for_core(r, order, yext)
            k16 = k16_interior
        else:
            k16 = k16_for_core(r, order, yext)
        hpix = (yext[:, None] * W + np.arange(W)[None, :]).reshape(-1)
        h = np.ascontiguousarray(H_all[:, hpix])
        icn = np.ascontiguousarray(
            logits.reshape(C, H, W)[:, RPC * r:RPC * (r + 1), :].reshape(C, NMID))
        offsets = np.array([[(PADBLK + 9 * r) * C,
                             9 * r * C,
                             (PADBLK + 9 * r + 9) * C]], np.uint32)
        in_maps.append({
            "g_feats": g,
            "h_feats": h,
            "k16_init": k16,
            "inp_pp": ipp,
            "inp_cn": icn,
            "offsets": offsets,
        })
    return in_maps


def _assemble(results):
    out = np.empty((C, N), np.float32)
    for r in range(NCORES):
        blk = results[r]["out_loc"].reshape(BLK, NMID // BLK, C)
        out[:, NMID * r:NMID * (r + 1)] = (
            blk.transpose(2, 1, 0).reshape(C, NMID))
    return out.reshape(1, C, H, W)


def _get_nc():
    global _CACHED_NC
    if _CACHED_NC is None:
        _CACHED_NC = _build_module()
    return _CACHED_NC


def run(input_tensor, reference_tensor, trace=False):
    from concourse.bass_utils import run_bass_kernel_spmd
    nc = _get_nc()
    in_maps = _host_inputs(input_tensor, reference_tensor)
    res = run_bass_kernel_spmd(nc, in_maps, core_ids=list(range(NCORES)),
                               trace=trace)
    return _assemble(res.results), res


def kernel(input_tensor, reference_tensor):
    out, _ = run(input_tensor, reference_tensor, trace=False)
    return out


# revision 24
# speedup vs baseline: 1.0071x; 1.0071x over previous
"""Dense-CRF mean-field inference on 8 Trainium2 NeuronCores.

Math restructuring (validated numerically against the jax reference):
  - Kb and Kg share the spatial sigma (5.0), so
        K = Kb + Kg = Kg * (1 + Cc),
    where Cc = exp(-.5||ci-cj||^2/sig_c^2) is a pure COLOR Gaussian.
    Only Cc is input-dependent; Kg (and the x3 UPDATE_FACTOR fold) is
    separable spatial structure the host precomputes as per-block
    rank-1 factors gy[128,14] (x) gx[128,96].
  - Color feature products are <= ~6 in magnitude -> the Cc feature
    matmul is fp16-safe; the whole K band lives in SBUF as fp16
    (1 PE cycle/row vs 4 for fp32). Simulated end-to-end rel err 5e-4
    vs the 2e-2 gate.
  - The Potts 3x3 conv update reduces to out = softmax(input +
    boxsum3(comb)) (class-independent part drops in softmax).
  - Band: 37 global 128-px blocks per core (sim: 37 -> 5e-4, 33 ->
    2.6e-2, so 37 is the minimum safe width). Per-core band order is
    [own 9 | left 14 | right 14] so runtime ds() offsets can split the
    flat copy; out-of-image blocks get gy=0 -> K=0.
  - One fp16 AllGather of the per-core probabilities per iteration.

Sharding: core r owns output image rows [12r, 12r+12).
"""

import os
import sys

import numpy as np

for _p in ("/opt/trn_rl_repo",):
    if _p not in sys.path and os.path.isdir(_p):
        sys.path.insert(0, _p)

H = 96
W = 96
C = 5
N = H * W                      # 9216
NCORES = 8
RPC = H // NCORES              # 12 image rows per core
EXT = RPC + 2                  # 14 rows incl. 1 halo row each side
NLOC = EXT * W                 # 1344 extended-output pixels
NMID = RPC * W                 # 1152 owned pixels
BLK = 128
NBLK = 37                      # K band m-blocks per core
HB = (NBLK - 9) // 2           # 14 blocks each side of the 9 own
GBLK = N // BLK                # 72 global blocks
PADBLK = HB                    # padding blocks each side of flat_pad
FPW = (GBLK + 2 * PADBLK) * C  # flat_pad free width = 500
# matvec n-chunks, row-aligned so the x-box can read PSUM directly
CHROWS = (5, 5, 4)
CHS = [r * W for r in CHROWS]  # (480, 480, 384)
CH0 = [sum(CHS[:j]) for j in range(3)]
BCH = 448                      # build n-chunk (fits one PSUM bank)
# band-local near-block positions (within +-11 blocks of the own window;
# sim: J=11 -> 1.6e-3 rel err, J=9 -> 2e-2). Far blocks use the
# constant-color-factor approximation (1 + mean Cc) * Kg, shipped direct.
NEAR = list(range(0, 9)) + list(range(12, 34))
NFAR = [i for i in range(NBLK) if i not in NEAR]
ITERS = 5

_CACHED_NC = None


def _build_module():
    import concourse.bass as bass
    import concourse.bacc as bacc
    import concourse.tile as tile
    from concourse import mybir
    from concourse.masks import make_identity

    f32 = mybir.dt.float32
    f16 = mybir.dt.float16
    u32 = mybir.dt.uint32
    EXP = mybir.ActivationFunctionType.Exp
    COPY = mybir.ActivationFunctionType.Copy
    ADD = mybir.AluOpType.add
    MULT = mybir.AluOpType.mult

    nc = bacc.Bacc("TRN2", target_bir_lowering=False, debug=False,
                   num_devices=NCORES)

    g_dram = nc.dram_tensor("g_feats", [C, len(NEAR) * BLK], f16,
                            kind="ExternalInput")
    h_dram = nc.dram_tensor("h_feats", [C, NLOC], f16, kind="ExternalInput")
    k16_dram = nc.dram_tensor("k16_init", [BLK, NBLK * NLOC], f16,
                              kind="ExternalInput")
    ipp_dram = nc.dram_tensor("inp_pp", [BLK, GBLK * C], f32, kind="ExternalInput")
    icn_dram = nc.dram_tensor("inp_cn", [C, NMID], f32, kind="ExternalInput")
    off_dram = nc.dram_tensor("offsets", [1, 3], u32, kind="ExternalInput")
    out_dram = nc.dram_tensor("out_loc", [BLK, (NMID // BLK) * C], f32,
                              kind="ExternalOutput")

    def bcast_inner(ap, n):
        return bass.AP(tensor=ap.tensor, offset=ap.offset, ap=[*ap.ap, [0, n]])

    def bcast_mid(ap, n):
        # [p, q] -> [p, n, q] with stride-0 middle dim
        return bass.AP(tensor=ap.tensor, offset=ap.offset,
                       ap=[ap.ap[0], [0, n], *ap.ap[1:]])

    with tile.TileContext(nc) as tc:
        with tc.tile_pool(name="singles", bufs=1) as singles, \
             tc.tile_pool(name="bpsum", bufs=2, space="PSUM") as bppool, \
             tc.tile_pool(name="ipsum", bufs=2, space="PSUM") as ippool, \
             tc.tile_pool(name="iter", bufs=1) as wpool, \
             tc.tile_pool(name="band", bufs=2) as bpool, \
             tc.tile_pool(name="smx", bufs=2) as spool, \
             tc.tile_pool(name="dram", bufs=1, space="DRAM") as dram:

            # ---- long-lived SBUF state ----
            k16 = singles.tile([BLK, NBLK, NLOC], f16, name="k16")
            flat_pad = singles.tile([BLK, FPW], f16, name="flat_pad")
            g_sb = singles.tile([C, len(NEAR) * BLK], f16, name="g_sb")
            h_sb = singles.tile([C, NLOC], f16, name="h_sb")
            ipp_sb = singles.tile([BLK, GBLK * C], f32, name="ipp_sb")
            icn_sb = singles.tile([C, NMID], f32, name="icn_sb")
            ident = singles.tile([BLK, BLK], f32, name="ident")
            off_sb = singles.tile([1, 3], u32, name="off_sb")

            nc.sync.dma_start(out=ipp_sb, in_=ipp_dram[:, :])
            nc.sync.dma_start(out=icn_sb, in_=icn_dram[:, :])
            nc.sync.dma_start(out=off_sb, in_=off_dram[:, :])
            nc.sync.dma_start(out=g_sb, in_=g_dram[:, :])
            nc.sync.dma_start(out=h_sb, in_=h_dram[:, :])
            # k16 initial values (spatial gaussian factors): per near block so
            # the build pipeline starts as soon as each block lands; far
            # blocks (2 contiguous runs) need no device work at all
            for i in NEAR:
                nc.sync.dma_start(
                    out=k16[:, i, :],
                    in_=k16_dram[:, i * NLOC:(i + 1) * NLOC])
            nc.sync.dma_start(
                out=k16[:, NFAR[0]:NFAR[2] + 1, :],
                in_=k16_dram[:, NFAR[0] * NLOC:(NFAR[2] + 1) * NLOC])
            nc.sync.dma_start(
                out=k16[:, NFAR[3]:NFAR[5] + 1, :],
                in_=k16_dram[:, NFAR[3] * NLOC:(NFAR[5] + 1) * NLOC])
            make_identity(nc, ident)
            nc.vector.memset(flat_pad, 0.0)

            # runtime flat_pad element offsets: own / left / right windows
            offs = []
            for j, mx in enumerate(((PADBLK + 9 * (NCORES - 1)) * C,
                                    (PADBLK + 9 * (NCORES - 1) - HB) * C,
                                    (PADBLK + 9 * (NCORES - 1) + 9) * C)):
                regs = nc.alloc_registers(f"off_regs{j}",
                                          engines=(mybir.EngineType.DVE,))
                nc.regs_load(regs, off_sb[0:1, j:j + 1])
                offs.append(nc.snap(regs, donate=True, min_val=0, max_val=mx))
            own_off, left_off, right_off = offs

            # HAM warm-keeper: fp16 matmuls (~213 ns each) that fill PE-idle
            # windows so the activity monitor keeps the PE clock at 2.4 GHz.
            # Each warm's lhsT reads an anchor tile produced just before the
            # idle window -- without the data dependency the static scheduler
            # hoists dep-free matmuls to the very start of the Tensor queue.
            def warm(n, anchor):
                wp = bppool.tile([BLK, 512], f32, tag="pb")
                for _ in range(n):
                    nc.tensor.matmul(wp[0:1, :], anchor[:, 0:1],
                                     k16[:, 0, 0:512], start=True, stop=True)

            def warm32(n, anchor):
                # fp32 anchor with few partitions: 128 fp32 cols = 512 cycles
                wp = bppool.tile([BLK, 512], f32, tag="pb")
                p = anchor.partition_size()
                for _ in range(n):
                    nc.tensor.matmul(wp[0:1, 0:BLK], anchor[:, 0:1],
                                     ident[0:p, 0:BLK], start=True, stop=True)

            ag_in = dram.tile([BLK, (NMID // BLK) * C], f16, name="ag_in")
            ag_out = dram.tile([BLK * NCORES, (NMID // BLK) * C], f16,
                               name="ag_out")
            cc_wu_in = dram.tile([BLK, 1], f16, name="cc_wu_in")
            cc_wu_out = dram.tile([BLK * NCORES, 1], f16, name="cc_wu_out")
            # warm up the collective rings during the build phase; the first
            # AllGather otherwise pays ~35us of one-time setup on the
            # critical path
            nc.gpsimd.collective_compute(
                "AllGather",
                mybir.AluOpType.bypass,
                replica_groups=[list(range(NCORES))],
                ins=[cc_wu_in.opt()],
                outs=[cc_wu_out.opt()],
            )

            # ---- helpers ----
            def softmax_pp(pool, u_pp, mb, tag, out_dt=f32):
                """u_pp: [128, mb*C] logits, pixel-partition layout -> probs."""
                v = u_pp.rearrange("p (a c) -> p a c", c=C)
                mx = pool.tile([BLK, mb], f32, tag=f"{tag}_mx")
                nc.vector.tensor_reduce(out=mx, in_=v,
                                        axis=mybir.AxisListType.X,
                                        op=mybir.AluOpType.max)
                e = pool.tile([BLK, mb * C], f32, tag=f"{tag}_e")
                ev = e.rearrange("p (a c) -> p a c", c=C)
                nc.vector.tensor_sub(ev, v, bcast_inner(mx, C))
                nc.scalar.activation(out=e, in_=e, func=EXP)
                s = pool.tile([BLK, mb], f32, tag=f"{tag}_s")
                nc.vector.tensor_reduce(out=s, in_=ev,
                                        axis=mybir.AxisListType.X,
                                        op=mybir.AluOpType.add)
                nc.vector.reciprocal(out=s, in_=s)
                fl = pool.tile([BLK, mb * C], out_dt, tag=f"{tag}_fl")
                nc.vector.tensor_mul(fl.rearrange("p (a c) -> p a c", c=C), ev,
                                     bcast_inner(s, C))
                return fl

            # ---- phase 2: initial flat = softmax(input), all cores alike ----
            with tc.tile_pool(name="init", bufs=1) as ipool:
                fl0 = softmax_pp(ipool, ipp_sb, GBLK, "sm0", out_dt=f16)
                nc.vector.tensor_copy(
                    out=flat_pad[:, PADBLK * C:(PADBLK + GBLK) * C], in_=fl0)

            # ---- phase 1: build the fp16 K band, fused with iteration
            # 0's matvec (block i's matmuls run right after k16[i] is
            # assembled, hiding iter-0's PE work under the vector-bound
            # build pipeline) ----
            with tc.tile_pool(name="kgstage", bufs=3) as kgpool:
                for j, i in enumerate(NEAR):
                    gt = g_sb[:, j * BLK:(j + 1) * BLK]
                    texp = kgpool.tile([BLK, NLOC], f16, tag="texp")
                    for nb in range(3):
                        pb = bppool.tile([BLK, 512], f32, tag="pb")
                        hs = h_sb[:, nb * BCH:(nb + 1) * BCH]
                        nc.tensor.matmul(pb[:, 0:BCH], gt, hs,
                                         start=True, stop=True)
                        nc.scalar.activation(
                            out=texp[:, nb * BCH:(nb + 1) * BCH],
                            in_=pb[:, 0:BCH], func=EXP)
                    ksl = k16[:, i, :]
                    nc.vector.scalar_tensor_tensor(
                        out=ksl, in0=texp, scalar=1.0, in1=ksl,
                        op0=ADD, op1=MULT)

            # second ring warm-up doubling as a cross-core barrier: absorbs
            # build-phase skew while iteration 0's matvec runs
            nc.gpsimd.collective_compute(
                "AllGather",
                mybir.AluOpType.bypass,
                replica_groups=[list(range(NCORES))],
                ins=[cc_wu_in.opt()],
                outs=[cc_wu_out.opt()],
            )

            # ---- phase 3: iterations ----
            pv_next = None
            for it in range(ITERS):
                # matvec: comb[c, n] = sum_m K[m, n] * flat[c, m]
                # for it > 0 the own 9 blocks were accumulated into
                # pv_next while the AllGather was in flight
                band = bpool.tile([BLK, NBLK * C], f16, tag="band")
                if it == 0:
                    pv = ippool.tile([BLK, 3, 512], f32, tag="pv")
                    nc.vector.tensor_copy(
                        out=band[:, 0:9 * C],
                        in_=flat_pad[:, bass.ds(own_off, 9 * C)])
                    i_lo = 0
                else:
                    pv = pv_next
                    i_lo = 9
                    nc.sync.dma_start(
                        out=flat_pad[:, PADBLK * C:(PADBLK + GBLK) * C]
                        .rearrange("p (r j) -> p r j", r=NCORES),
                        in_=ag_out.rearrange("(r p) j -> p r j", p=BLK))
                nc.vector.tensor_copy(
                    out=band[:, 9 * C:(9 + HB) * C],
                    in_=flat_pad[:, bass.ds(left_off, HB * C)])
                nc.vector.tensor_copy(
                    out=band[:, (9 + HB) * C:NBLK * C],
                    in_=flat_pad[:, bass.ds(right_off, HB * C)])
                for nb in range(3):
                    for i in range(i_lo, NBLK):
                        nc.tensor.matmul(
                            pv[0:C, nb, 0:CHS[nb]],
                            band[:, i * C:(i + 1) * C],
                            k16[:, i, CH0[nb]:CH0[nb] + CHS[nb]],
                            start=(it == 0 and i == 0),
                            stop=(i == NBLK - 1))

                # 3x3 box sum, x-pass per row-aligned PSUM chunk
                # (hardware: at most one PSUM operand per vector op, so
                # seed t1 with a scalar-engine copy then accumulate);
                # y-pass and transposes are chunk-pipelined so only
                # chunk 2's tail is serial after the matvec. The
                # transposed logits land in chunk 0's already-consumed
                # PSUM bank (pv[:, 0, 0:45]).
                t1 = wpool.tile([C, NLOC], f32, tag="t1")
                t1r = t1.rearrange("p (row x) -> p row x", x=W)
                u = wpool.tile([C, NMID], f32, tag="u")
                ptv = pv[:, 0, 0:45]
                r0 = 0
                for nb in range(3):
                    nr = CHROWS[nb]
                    cb = pv[0:C, nb, 0:CHS[nb]].rearrange(
                        "p (row x) -> p row x", x=W)
                    tb = t1r[:, r0:r0 + nr, :]
                    nc.scalar.activation(out=tb, in_=cb, func=COPY)
                    nc.vector.tensor_add(tb[:, :, 1:W - 1],
                                         tb[:, :, 1:W - 1],
                                         cb[:, :, 0:W - 2])
                    nc.vector.tensor_add(tb[:, :, 1:W - 1],
                                         tb[:, :, 1:W - 1],
                                         cb[:, :, 2:W])
                    nc.vector.tensor_add(tb[:, :, 0:1], tb[:, :, 0:1],
                                         cb[:, :, 0:1])
                    nc.vector.tensor_add(tb[:, :, 0:1], tb[:, :, 0:1],
                                         cb[:, :, 1:2])
                    nc.vector.tensor_add(tb[:, :, W - 1:W],
                                         tb[:, :, W - 1:W],
                                         cb[:, :, W - 1:W])
                    nc.vector.tensor_add(tb[:, :, W - 1:W],
                                         tb[:, :, W - 1:W],
                                         cb[:, :, W - 2:W - 1])
                    r0 += nr
                    if nb == 0:
                        continue
                    if nb == 1:
                        # u rows 0-7 (needs t1 rows 0-9) + transposes 0-5
                        a, b = 0, 8 * W
                    else:
                        # u rows 8-11 (needs t1 rows 8-13) + transposes 6-8
                        a, b = 8 * W, NMID
                    nc.vector.tensor_add(u[:, a:b], t1[:, a:b],
                                         t1[:, a + 2 * W:b + 2 * W])
                    nc.vector.tensor_add(u[:, a:b], u[:, a:b],
                                         t1[:, a + W:b + W])
                    nc.vector.tensor_add(u[:, a:b], u[:, a:b],
                                         icn_sb[:, a:b])
                    for kb in range(a // BLK, b // BLK):
                        nc.tensor.transpose(ptv[:, kb * C:(kb + 1) * C],
                                            u[:, kb * BLK:(kb + 1) * BLK],
                                            ident[0:C, 0:C])
                    warm32(4, u)

                if it < ITERS - 1:
                    flat_l = softmax_pp(spool, ptv, NMID // BLK, "smx",
                                        out_dt=f16)
                    nc.sync.dma_start(out=ag_in, in_=flat_l)
                    nc.gpsimd.collective_compute(
                        "AllGather",
                        mybir.AluOpType.bypass,
                        replica_groups=[list(range(NCORES))],
                        ins=[ag_in.opt()],
                        outs=[ag_out.opt()],
                    )
                    # overlap the AllGather with next iteration's own-block
                    # matmul accumulation (flat_l is this core's own data)
                    pv_next = ippool.tile([BLK, 3, 512], f32, tag="pv")
                    for nb in range(3):
                        for j in range(9):
                            nc.tensor.matmul(
                                pv_next[0:C, nb, 0:CHS[nb]],
                                flat_l[:, j * C:(j + 1) * C],
                                k16[:, j, CH0[nb]:CH0[nb] + CHS[nb]],
                                start=(j == 0), stop=False)
                    warm(34, flat_l)
                else:
                    flat_l = softmax_pp(spool, ptv, NMID // BLK, "smx",
                                        out_dt=f32)
                    nc.sync.dma_start(out=out_dram[:, :], in_=flat_l)

    nc.compile()
    return nc


def _host_inputs(input_tensor, reference_tensor):
    logits = np.ascontiguousarray(
        np.asarray(input_tensor, dtype=np.float32)[0].reshape(C, N))
    ref = np.asarray(reference_tensor, dtype=np.float32)[0]  # [3, 96, 96]

    RGB = (ref / 0.5).reshape(3, N).astype(np.float32)
    c2 = (-0.5 * (RGB * RGB).sum(axis=0)).astype(np.float32)
    ones = np.ones(N, np.float32)
    G_all = np.stack([RGB[0], RGB[1], RGB[2], c2, ones]).astype(np.float16)
    H_all = np.stack([RGB[0], RGB[1], RGB[2], ones, c2]).astype(np.float16)

    # mean color-kernel value for the far-block constant approximation
    samp = RGB[:, ::37]
    d2 = ((samp[:, :, None] - samp[:, None, :]) ** 2).sum(axis=0)
    tbar = float(np.exp(-0.5 * d2).mean())

    # input in pixel-partition layout [128, 72*5]
    ipp = np.ascontiguousarray(
        logits.reshape(C, GBLK, BLK).transpose(2, 1, 0).reshape(BLK, GBLK * C))

    # spatial gaussian tables; x table carries the 3.0 UPDATE_FACTOR fold
    dtab = np.exp(-(np.arange(-(H - 1), H) ** 2) / 50.0)
    gy1 = dtab.astype(np.float32)
    gx3 = (3.0 * dtab).astype(np.float32)
    yy_all = (np.arange(N) // W).astype(np.int64)
    xx_all = (np.arange(N) % W).astype(np.int64)

    def k16_for_core(r, order, yext):
        k = np.zeros((BLK, NBLK, NLOC), np.float16)
        xn = np.arange(W)
        for i, gb in enumerate(order):
            if 0 <= gb < GBLK:
                pm = np.arange(gb * BLK, (gb + 1) * BLK)
                A = gy1[yy_all[pm][:, None] - yext[None, :] + H - 1]
                B = gx3[xx_all[pm][:, None] - xn[None, :] + H - 1]
                kg = (A[:, :, None] * B[:, None, :]).reshape(BLK, NLOC)
                if i in NFAR:
                    kg *= (1.0 + tbar)
                k[:, i, :] = kg.astype(np.float16)
        return k.reshape(BLK, NBLK * NLOC)

    in_maps = []
    k16_interior = None
    for r in range(NCORES):
        # band-local order: [own 9 | left 14 | right 14] global blocks
        order = (list(range(9 * r, 9 * r + 9))
                 + list(range(9 * r - HB, 9 * r))
                 + list(range(9 * r + 9, 9 * r + 9 + HB)))
        yext = np.clip(np.arange(RPC * r - 1, RPC * (r + 1) + 1), 0, H - 1)
        g = np.zeros((C, len(NEAR) * BLK), np.float16)
        for j, i in enumerate(NEAR):
            gb = order[i]
            if 0 <= gb < GBLK:
                g[:, j * BLK:(j + 1) * BLK] = G_all[:, gb * BLK:(gb + 1) * BLK]
        if 2 <= r <= 5:
            if k16_interior is None:
                k16_interior = k16_for_core(r, order, yext)
            k16 = k16_interior
        else:
            k16 = k16_for_core(r, order, yext)
        hpix = (yext[:, None] * W + np.arange(W)[None, :]).reshape(-1)
        h = np.ascontiguousarray(H_all[:, hpix])
        icn = np.ascontiguousarray(
            logits.reshape(C, H, W)[:, RPC * r:RPC * (r + 1), :].reshape(C, NMID))
        offsets = np.array([[(PADBLK + 9 * r) * C,
                             9 * r * C,
                             (PADBLK + 9 * r + 9) * C]], np.uint32)
        in_maps.append({
            "g_feats": g,
            "h_feats": h,
            "k16_init": k16,
            "inp_pp": ipp,
            "inp_cn": icn,
            "offsets": offsets,
        })
    return in_maps


def _assemble(results):
    out = np.empty((C, N), np.float32)
    for r in range(NCORES):
        blk = results[r]["out_loc"].reshape(BLK, NMID // BLK, C)
        out[:, NMID * r:NMID * (r + 1)] = (
            blk.transpose(2, 1, 0).reshape(C, NMID))
    return out.reshape(1, C, H, W)


def _get_nc():
    global _CACHED_NC
    if _CACHED_NC is None:
        _CACHED_NC = _build_module()
    return _CACHED_NC


def run(input_tensor, reference_tensor, trace=False):
    from concourse.bass_utils import run_bass_kernel_spmd
    nc = _get_nc()
    in_maps = _host_inputs(input_tensor, reference_tensor)
    res = run_bass_kernel_spmd(nc, in_maps, core_ids=list(range(NCORES)),
                               trace=trace)
    return _assemble(res.results), res


def kernel(input_tensor, reference_tensor):
    out, _ = run(input_tensor, reference_tensor, trace=False)
    return out


# revision 25
# speedup vs baseline: 1.0382x; 1.0308x over previous
"""Dense-CRF mean-field inference on 8 Trainium2 NeuronCores.

Math restructuring (validated numerically against the jax reference):
  - Kb and Kg share the spatial sigma (5.0), so
        K = Kb + Kg = Kg * (1 + Cc),
    where Cc = exp(-.5||ci-cj||^2/sig_c^2) is a pure COLOR Gaussian.
    Only Cc is input-dependent; Kg (and the x3 UPDATE_FACTOR fold) is
    separable spatial structure the host precomputes as per-block
    rank-1 factors gy[128,14] (x) gx[128,96].
  - Color feature products are <= ~6 in magnitude -> the Cc feature
    matmul is fp16-safe; the whole K band lives in SBUF as fp16
    (1 PE cycle/row vs 4 for fp32). Simulated end-to-end rel err 5e-4
    vs the 2e-2 gate.
  - The Potts 3x3 conv update reduces to out = softmax(input +
    boxsum3(comb)) (class-independent part drops in softmax).
  - Band: 37 global 128-px blocks per core (sim: 37 -> 5e-4, 33 ->
    2.6e-2, so 37 is the minimum safe width). Per-core band order is
    [own 9 | left 14 | right 14] so runtime ds() offsets can split the
    flat copy; out-of-image blocks get gy=0 -> K=0.
  - One fp16 AllGather of the per-core probabilities per iteration.

Sharding: core r owns output image rows [12r, 12r+12).
"""

import os
import sys

import numpy as np

for _p in ("/opt/trn_rl_repo",):
    if _p not in sys.path and os.path.isdir(_p):
        sys.path.insert(0, _p)

H = 96
W = 96
C = 5
N = H * W                      # 9216
NCORES = 8
RPC = H // NCORES              # 12 image rows per core
EXT = RPC + 2                  # 14 rows incl. 1 halo row each side
NLOC = EXT * W                 # 1344 extended-output pixels
NMID = RPC * W                 # 1152 owned pixels
BLK = 128
NBLK = 37                      # K band m-blocks per core
HB = (NBLK - 9) // 2           # 14 blocks each side of the 9 own
GBLK = N // BLK                # 72 global blocks
PADBLK = HB                    # padding blocks each side of flat_pad
FPW = (GBLK + 2 * PADBLK) * C  # flat_pad free width = 500
# matvec n-chunks, row-aligned so the x-box can read PSUM directly
CHROWS = (5, 5, 4)
CHS = [r * W for r in CHROWS]  # (480, 480, 384)
CH0 = [sum(CHS[:j]) for j in range(3)]
BCH = 448                      # build n-chunk (fits one PSUM bank)
# band-local near-block positions (within +-11 blocks of the own window;
# sim: J=11 -> 1.6e-3 rel err, J=9 -> 2e-2). Far blocks use the
# constant-color-factor approximation (1 + mean Cc) * Kg, shipped direct.
NEAR = list(range(0, 9)) + list(range(12, 34))
NFAR = [i for i in range(NBLK) if i not in NEAR]
ITERS = 5

_CACHED_NC = None


def _build_module():
    import concourse.bass as bass
    import concourse.bacc as bacc
    import concourse.tile as tile
    from concourse import mybir
    from concourse.masks import make_identity

    f32 = mybir.dt.float32
    f16 = mybir.dt.float16
    u32 = mybir.dt.uint32
    EXP = mybir.ActivationFunctionType.Exp
    COPY = mybir.ActivationFunctionType.Copy
    ADD = mybir.AluOpType.add
    MULT = mybir.AluOpType.mult

    nc = bacc.Bacc("TRN2", target_bir_lowering=False, debug=False,
                   num_devices=NCORES)

    g_dram = nc.dram_tensor("g_feats", [C, len(NEAR) * BLK], f16,
                            kind="ExternalInput")
    h_dram = nc.dram_tensor("h_feats", [C, NLOC], f16, kind="ExternalInput")
    k16_dram = nc.dram_tensor("k16_init", [BLK, NBLK * NLOC], f16,
                              kind="ExternalInput")
    ipp_dram = nc.dram_tensor("inp_pp", [BLK, GBLK * C], f32, kind="ExternalInput")
    icn_dram = nc.dram_tensor("inp_cn", [C, NMID], f32, kind="ExternalInput")
    off_dram = nc.dram_tensor("offsets", [1, 3], u32, kind="ExternalInput")
    out_dram = nc.dram_tensor("out_loc", [BLK, (NMID // BLK) * C], f32,
                              kind="ExternalOutput")

    def bcast_inner(ap, n):
        return bass.AP(tensor=ap.tensor, offset=ap.offset, ap=[*ap.ap, [0, n]])

    def bcast_mid(ap, n):
        # [p, q] -> [p, n, q] with stride-0 middle dim
        return bass.AP(tensor=ap.tensor, offset=ap.offset,
                       ap=[ap.ap[0], [0, n], *ap.ap[1:]])

    with tile.TileContext(nc) as tc:
        with tc.tile_pool(name="singles", bufs=1) as singles, \
             tc.tile_pool(name="bpsum", bufs=2, space="PSUM") as bppool, \
             tc.tile_pool(name="ipsum", bufs=2, space="PSUM") as ippool, \
             tc.tile_pool(name="iter", bufs=1) as wpool, \
             tc.tile_pool(name="band", bufs=2) as bpool, \
             tc.tile_pool(name="smx", bufs=2) as spool, \
             tc.tile_pool(name="dram", bufs=1, space="DRAM") as dram:

            # ---- long-lived SBUF state ----
            k16 = singles.tile([BLK, NBLK, NLOC], f16, name="k16")
            flat_pad = singles.tile([BLK, FPW], f16, name="flat_pad")
            g_sb = singles.tile([C, len(NEAR) * BLK], f16, name="g_sb")
            h_sb = singles.tile([C, NLOC], f16, name="h_sb")
            ipp_sb = singles.tile([BLK, GBLK * C], f32, name="ipp_sb")
            icn_sb = singles.tile([C, NMID], f32, name="icn_sb")
            ident = singles.tile([BLK, BLK], f32, name="ident")
            off_sb = singles.tile([1, 3], u32, name="off_sb")

            nc.sync.dma_start(out=ipp_sb, in_=ipp_dram[:, :])
            nc.sync.dma_start(out=icn_sb, in_=icn_dram[:, :])
            nc.sync.dma_start(out=off_sb, in_=off_dram[:, :])
            nc.sync.dma_start(out=g_sb, in_=g_dram[:, :])
            nc.sync.dma_start(out=h_sb, in_=h_dram[:, :])
            # k16 initial values (spatial gaussian factors): per near block so
            # the build pipeline starts as soon as each block lands; far
            # blocks (2 contiguous runs) need no device work at all
            for i in NEAR:
                nc.sync.dma_start(
                    out=k16[:, i, :],
                    in_=k16_dram[:, i * NLOC:(i + 1) * NLOC])
            nc.sync.dma_start(
                out=k16[:, NFAR[0]:NFAR[2] + 1, :],
                in_=k16_dram[:, NFAR[0] * NLOC:(NFAR[2] + 1) * NLOC])
            nc.sync.dma_start(
                out=k16[:, NFAR[3]:NFAR[5] + 1, :],
                in_=k16_dram[:, NFAR[3] * NLOC:(NFAR[5] + 1) * NLOC])
            make_identity(nc, ident)
            nc.vector.memset(flat_pad, 0.0)

            # runtime flat_pad element offsets: own / left / right windows
            offs = []
            for j, mx in enumerate(((PADBLK + 9 * (NCORES - 1)) * C,
                                    (PADBLK + 9 * (NCORES - 1) - HB) * C,
                                    (PADBLK + 9 * (NCORES - 1) + 9) * C)):
                regs = nc.alloc_registers(f"off_regs{j}",
                                          engines=(mybir.EngineType.DVE,))
                nc.regs_load(regs, off_sb[0:1, j:j + 1])
                offs.append(nc.snap(regs, donate=True, min_val=0, max_val=mx))
            own_off, left_off, right_off = offs

            # HAM warm-keeper: fp16 matmuls (~213 ns each) that fill PE-idle
            # windows so the activity monitor keeps the PE clock at 2.4 GHz.
            # Each warm's lhsT reads an anchor tile produced just before the
            # idle window -- without the data dependency the static scheduler
            # hoists dep-free matmuls to the very start of the Tensor queue.
            def warm(n, anchor):
                wp = bppool.tile([BLK, 512], f32, tag="pb")
                for _ in range(n):
                    nc.tensor.matmul(wp[0:1, :], anchor[:, 0:1],
                                     k16[:, 0, 0:512], start=True, stop=True)

            def warm32(n, anchor):
                # fp32 anchor with few partitions: 128 fp32 cols = 512 cycles
                wp = bppool.tile([BLK, 512], f32, tag="pb")
                p = anchor.partition_size()
                for _ in range(n):
                    nc.tensor.matmul(wp[0:1, 0:BLK], anchor[:, 0:1],
                                     ident[0:p, 0:BLK], start=True, stop=True)

            ag_in = dram.tile([BLK, (NMID // BLK) * C], f16, name="ag_in")
            ag_out = dram.tile([BLK * NCORES, (NMID // BLK) * C], f16,
                               name="ag_out")
            cc_wu_in = dram.tile([BLK, 1], f16, name="cc_wu_in")
            cc_wu_out = dram.tile([BLK * NCORES, 1], f16, name="cc_wu_out")
            # warm up the collective rings during the build phase; the first
            # AllGather otherwise pays ~35us of one-time setup on the
            # critical path
            nc.gpsimd.collective_compute(
                "AllGather",
                mybir.AluOpType.bypass,
                replica_groups=[list(range(NCORES))],
                ins=[cc_wu_in.opt()],
                outs=[cc_wu_out.opt()],
            )

            # ---- helpers ----
            def softmax_pp(pool, u_pp, mb, tag, out_dt=f32):
                """u_pp: [128, mb*C] logits, pixel-partition layout -> probs."""
                v = u_pp.rearrange("p (a c) -> p a c", c=C)
                mx = pool.tile([BLK, mb], f32, tag=f"{tag}_mx")
                nc.vector.tensor_reduce(out=mx, in_=v,
                                        axis=mybir.AxisListType.X,
                                        op=mybir.AluOpType.max)
                e = pool.tile([BLK, mb * C], f32, tag=f"{tag}_e")
                ev = e.rearrange("p (a c) -> p a c", c=C)
                nc.vector.tensor_sub(ev, v, bcast_inner(mx, C))
                nc.scalar.activation(out=e, in_=e, func=EXP)
                s = pool.tile([BLK, mb], f32, tag=f"{tag}_s")
                nc.vector.tensor_reduce(out=s, in_=ev,
                                        axis=mybir.AxisListType.X,
                                        op=mybir.AluOpType.add)
                nc.vector.reciprocal(out=s, in_=s)
                fl = pool.tile([BLK, mb * C], out_dt, tag=f"{tag}_fl")
                nc.vector.tensor_mul(fl.rearrange("p (a c) -> p a c", c=C), ev,
                                     bcast_inner(s, C))
                return fl

            # ---- phase 2: initial flat = softmax(input), all cores alike ----
            with tc.tile_pool(name="init", bufs=1) as ipool:
                fl0 = softmax_pp(ipool, ipp_sb, GBLK, "sm0", out_dt=f16)
                nc.vector.tensor_copy(
                    out=flat_pad[:, PADBLK * C:(PADBLK + GBLK) * C], in_=fl0)

            # ---- phase 1: build the fp16 K band, fused with iteration
            # 0's matvec (block i's matmuls run right after k16[i] is
            # assembled, hiding iter-0's PE work under the vector-bound
            # build pipeline) ----
            with tc.tile_pool(name="kgstage", bufs=3) as kgpool:
                for j, i in enumerate(NEAR):
                    gt = g_sb[:, j * BLK:(j + 1) * BLK]
                    texp = kgpool.tile([BLK, NLOC], f16, tag="texp")
                    for nb in range(3):
                        pb = bppool.tile([BLK, 512], f32, tag="pb")
                        hs = h_sb[:, nb * BCH:(nb + 1) * BCH]
                        nc.tensor.matmul(pb[:, 0:BCH], gt, hs,
                                         start=True, stop=True)
                        nc.scalar.activation(
                            out=texp[:, nb * BCH:(nb + 1) * BCH],
                            in_=pb[:, 0:BCH], func=EXP)
                    ksl = k16[:, i, :]
                    nc.vector.scalar_tensor_tensor(
                        out=ksl, in0=texp, scalar=1.0, in1=ksl,
                        op0=ADD, op1=MULT)

            # second ring warm-up doubling as a cross-core barrier: absorbs
            # build-phase skew while iteration 0's matvec runs
            nc.gpsimd.collective_compute(
                "AllGather",
                mybir.AluOpType.bypass,
                replica_groups=[list(range(NCORES))],
                ins=[cc_wu_in.opt()],
                outs=[cc_wu_out.opt()],
            )

            # ---- phase 3: iterations ----
            pv_next = None
            for it in range(ITERS):
                # matvec: comb[c, n] = sum_m K[m, n] * flat[c, m]
                # for it > 0 the own 9 blocks were accumulated into
                # pv_next while the AllGather was in flight
                band = bpool.tile([BLK, NBLK * C], f16, tag="band")
                if it == 0:
                    pv = ippool.tile([BLK, 3, 512], f32, tag="pv")
                    nc.vector.tensor_copy(
                        out=band[:, 0:9 * C],
                        in_=flat_pad[:, bass.ds(own_off, 9 * C)])
                    i_lo = 0
                else:
                    pv = pv_next
                    i_lo = 9
                    nc.sync.dma_start(
                        out=flat_pad[:, PADBLK * C:(PADBLK + GBLK) * C]
                        .rearrange("p (r j) -> p r j", r=NCORES),
                        in_=ag_out.rearrange("(r p) j -> p r j", p=BLK))
                nc.vector.tensor_copy(
                    out=band[:, 9 * C:(9 + HB) * C],
                    in_=flat_pad[:, bass.ds(left_off, HB * C)])
                nc.vector.tensor_copy(
                    out=band[:, (9 + HB) * C:NBLK * C],
                    in_=flat_pad[:, bass.ds(right_off, HB * C)])
                for nb in range(3):
                    for i in range(i_lo, NBLK):
                        nc.tensor.matmul(
                            pv[0:C, nb, 0:CHS[nb]],
                            band[:, i * C:(i + 1) * C],
                            k16[:, i, CH0[nb]:CH0[nb] + CHS[nb]],
                            start=(it == 0 and i == 0),
                            stop=(i == NBLK - 1))

                # 3x3 box sum, x-pass per row-aligned PSUM chunk
                # (hardware: at most one PSUM operand per vector op, so
                # seed t1 with a scalar-engine copy then accumulate);
                # y-pass and transposes are chunk-pipelined so only
                # chunk 2's tail is serial after the matvec. The
                # transposed logits land in chunk 0's already-consumed
                # PSUM bank (pv[:, 0, 0:45]).
                t1 = wpool.tile([C, NLOC], f32, tag="t1")
                t1r = t1.rearrange("p (row x) -> p row x", x=W)
                u = wpool.tile([C, NMID], f32, tag="u")
                ptv = pv[:, 0, 0:45]
                r0 = 0
                for nb in range(3):
                    nr = CHROWS[nb]
                    cb = pv[0:C, nb, 0:CHS[nb]].rearrange(
                        "p (row x) -> p row x", x=W)
                    tb = t1r[:, r0:r0 + nr, :]
                    nc.scalar.activation(out=tb, in_=cb, func=COPY)
                    nc.vector.tensor_add(tb[:, :, 1:W - 1],
                                         tb[:, :, 1:W - 1],
                                         cb[:, :, 0:W - 2])
                    nc.vector.tensor_add(tb[:, :, 1:W - 1],
                                         tb[:, :, 1:W - 1],
                                         cb[:, :, 2:W])
                    nc.vector.tensor_add(tb[:, :, 0:1], tb[:, :, 0:1],
                                         cb[:, :, 0:1])
                    nc.vector.tensor_add(tb[:, :, 0:1], tb[:, :, 0:1],
                                         cb[:, :, 1:2])
                    nc.vector.tensor_add(tb[:, :, W - 1:W],
                                         tb[:, :, W - 1:W],
                                         cb[:, :, W - 1:W])
                    nc.vector.tensor_add(tb[:, :, W - 1:W],
                                         tb[:, :, W - 1:W],
                                         cb[:, :, W - 2:W - 1])
                    r0 += nr
                    if nb == 0:
                        continue
                    if nb == 1:
                        # u rows 0-7 (needs t1 rows 0-9) + transposes 0-5
                        a, b = 0, 8 * W
                    else:
                        # u rows 8-11 (needs t1 rows 8-13) + transposes 6-8
                        a, b = 8 * W, NMID
                    nc.vector.tensor_add(u[:, a:b], t1[:, a:b],
                                         t1[:, a + 2 * W:b + 2 * W])
                    nc.vector.tensor_add(u[:, a:b], u[:, a:b],
                                         t1[:, a + W:b + W])
                    nc.vector.tensor_add(u[:, a:b], u[:, a:b],
                                         icn_sb[:, a:b])
                    for kb in range(a // BLK, b // BLK):
                        nc.tensor.transpose(ptv[:, kb * C:(kb + 1) * C],
                                            u[:, kb * BLK:(kb + 1) * BLK],
                                            ident[0:C, 0:C])
                    warm32(4, u)

                if it < ITERS - 1:
                    flat_l = softmax_pp(spool, ptv, NMID // BLK, "smx",
                                        out_dt=f16)
                    nc.sync.dma_start(out=ag_in, in_=flat_l)
                    nc.gpsimd.collective_compute(
                        "AllGather",
                        mybir.AluOpType.bypass,
                        replica_groups=[list(range(NCORES))],
                        ins=[ag_in.opt()],
                        outs=[ag_out.opt()],
                    )
                    # overlap the AllGather with next iteration's own-block
                    # matmul accumulation (flat_l is this core's own data)
                    pv_next = ippool.tile([BLK, 3, 512], f32, tag="pv")
                    for nb in range(3):
                        for j in range(9):
                            nc.tensor.matmul(
                                pv_next[0:C, nb, 0:CHS[nb]],
                                flat_l[:, j * C:(j + 1) * C],
                                k16[:, j, CH0[nb]:CH0[nb] + CHS[nb]],
                                start=(j == 0), stop=False)
                    warm(22, flat_l)
                else:
                    flat_l = softmax_pp(spool, ptv, NMID // BLK, "smx",
                                        out_dt=f32)
                    nc.sync.dma_start(out=out_dram[:, :], in_=flat_l)

    nc.compile()
    return nc


def _host_inputs(input_tensor, reference_tensor):
    logits = np.ascontiguousarray(
        np.asarray(input_tensor, dtype=np.float32)[0].reshape(C, N))
    ref = np.asarray(reference_tensor, dtype=np.float32)[0]  # [3, 96, 96]

    RGB = (ref / 0.5).reshape(3, N).astype(np.float32)
    c2 = (-0.5 * (RGB * RGB).sum(axis=0)).astype(np.float32)
    ones = np.ones(N, np.float32)
    G_all = np.stack([RGB[0], RGB[1], RGB[2], c2, ones]).astype(np.float16)
    H_all = np.stack([RGB[0], RGB[1], RGB[2], ones, c2]).astype(np.float16)

    # mean color-kernel value for the far-block constant approximation
    samp = RGB[:, ::37]
    d2 = ((samp[:, :, None] - samp[:, None, :]) ** 2).sum(axis=0)
    tbar = float(np.exp(-0.5 * d2).mean())

    # input in pixel-partition layout [128, 72*5]
    ipp = np.ascontiguousarray(
        logits.reshape(C, GBLK, BLK).transpose(2, 1, 0).reshape(BLK, GBLK * C))

    # spatial gaussian tables; x table carries the 3.0 UPDATE_FACTOR fold
    dtab = np.exp(-(np.arange(-(H - 1), H) ** 2) / 50.0)
    gy1 = dtab.astype(np.float32)
    gx3 = (3.0 * dtab).astype(np.float32)
    yy_all = (np.arange(N) // W).astype(np.int64)
    xx_all = (np.arange(N) % W).astype(np.int64)

    def k16_for_core(r, order, yext):
        k = np.zeros((BLK, NBLK, NLOC), np.float16)
        xn = np.arange(W)
        for i, gb in enumerate(order):
            if 0 <= gb < GBLK:
                pm = np.arange(gb * BLK, (gb + 1) * BLK)
                A = gy1[yy_all[pm][:, None] - yext[None, :] + H - 1]
                B = gx3[xx_all[pm][:, None] - xn[None, :] + H - 1]
                kg = (A[:, :, None] * B[:, None, :]).reshape(BLK, NLOC)
                if i in NFAR:
                    kg *= (1.0 + tbar)
                k[:, i, :] = kg.astype(np.float16)
        return k.reshape(BLK, NBLK * NLOC)

    in_maps = []
    k16_interior = None
    for r in range(NCORES):
        # band-local order: [own 9 | left 14 | right 14] global blocks
        order = (list(range(9 * r, 9 * r + 9))
                 + list(range(9 * r - HB, 9 * r))
                 + list(range(9 * r + 9, 9 * r + 9 + HB)))
        yext = np.clip(np.arange(RPC * r - 1, RPC * (r + 1) + 1), 0, H - 1)
        g = np.zeros((C, len(NEAR) * BLK), np.float16)
        for j, i in enumerate(NEAR):
            gb = order[i]
            if 0 <= gb < GBLK:
                g[:, j * BLK:(j + 1) * BLK] = G_all[:, gb * BLK:(gb + 1) * BLK]
        if 2 <= r <= 5:
            if k16_interior is None:
                k16_interior = k16_for_core(r, order, yext)
            k16 = k16_interior
        else:
            k16 = k16_for_core(r, order, yext)
        hpix = (yext[:, None] * W + np.arange(W)[None, :]).reshape(-1)
        h = np.ascontiguousarray(H_all[:, hpix])
        icn = np.ascontiguousarray(
            logits.reshape(C, H, W)[:, RPC * r:RPC * (r + 1), :].reshape(C, NMID))
        offsets = np.array([[(PADBLK + 9 * r) * C,
                             9 * r * C,
                             (PADBLK + 9 * r + 9) * C]], np.uint32)
        in_maps.append({
            "g_feats": g,
            "h_feats": h,
            "k16_init": k16,
            "inp_pp": ipp,
            "inp_cn": icn,
            "offsets": offsets,
        })
    return in_maps


def _assemble(results):
    out = np.empty((C, N), np.float32)
    for r in range(NCORES):
        blk = results[r]["out_loc"].reshape(BLK, NMID // BLK, C)
        out[:, NMID * r:NMID * (r + 1)] = (
            blk.transpose(2, 1, 0).reshape(C, NMID))
    return out.reshape(1, C, H, W)


def _get_nc():
    global _CACHED_NC
    if _CACHED_NC is None:
        _CACHED_NC = _build_module()
    return _CACHED_NC


def run(input_tensor, reference_tensor, trace=False):
    from concourse.bass_utils import run_bass_kernel_spmd
    nc = _get_nc()
    in_maps = _host_inputs(input_tensor, reference_tensor)
    res = run_bass_kernel_spmd(nc, in_maps, core_ids=list(range(NCORES)),
                               trace=trace)
    return _assemble(res.results), res


def kernel(input_tensor, reference_tensor):
    out, _ = run(input_tensor, reference_tensor, trace=False)
    return out


# revision 26
# speedup vs baseline: 1.1301x; 1.0885x over previous
"""Dense-CRF mean-field inference on 8 Trainium2 NeuronCores.

Math restructuring (validated numerically against the jax reference):
  - Kb and Kg share the spatial sigma (5.0), so
        K = Kb + Kg = Kg * (1 + Cc),
    where Cc = exp(-.5||ci-cj||^2/sig_c^2) is a pure COLOR Gaussian.
    Only Cc is input-dependent; Kg (and the x3 UPDATE_FACTOR fold) is
    separable spatial structure the host precomputes as per-block
    rank-1 factors gy[128,14] (x) gx[128,96].
  - Color feature products are <= ~6 in magnitude -> the Cc feature
    matmul is fp16-safe; the whole K band lives in SBUF as fp16
    (1 PE cycle/row vs 4 for fp32). Simulated end-to-end rel err 5e-4
    vs the 2e-2 gate.
  - The Potts 3x3 conv update reduces to out = softmax(input +
    boxsum3(comb)) (class-independent part drops in softmax).
  - Band: 37 global 128-px blocks per core (sim: 37 -> 5e-4, 33 ->
    2.6e-2, so 37 is the minimum safe width). Per-core band order is
    [own 9 | left 14 | right 14] so runtime ds() offsets can split the
    flat copy; out-of-image blocks get gy=0 -> K=0.
  - One fp16 AllGather of the per-core probabilities per iteration.

Sharding: core r owns output image rows [12r, 12r+12).
"""

import os
import sys

import numpy as np

for _p in ("/opt/trn_rl_repo",):
    if _p not in sys.path and os.path.isdir(_p):
        sys.path.insert(0, _p)

H = 96
W = 96
C = 5
N = H * W                      # 9216
NCORES = 8
RPC = H // NCORES              # 12 image rows per core
EXT = RPC + 2                  # 14 rows incl. 1 halo row each side
NLOC = EXT * W                 # 1344 extended-output pixels
NMID = RPC * W                 # 1152 owned pixels
BLK = 128
NBLK = 37                      # K band m-blocks per core
HB = (NBLK - 9) // 2           # 14 blocks each side of the 9 own
GBLK = N // BLK                # 72 global blocks
PADBLK = HB                    # padding blocks each side of flat_pad
FPW = (GBLK + 2 * PADBLK) * C  # flat_pad free width = 500
# matvec n-chunks, row-aligned so the x-box can read PSUM directly
CHROWS = (5, 5, 4)
CHS = [r * W for r in CHROWS]  # (480, 480, 384)
CH0 = [sum(CHS[:j]) for j in range(3)]
BCH = 448                      # build n-chunk (fits one PSUM bank)
# band-local near-block positions (within +-11 blocks of the own window;
# sim: J=11 -> 1.6e-3 rel err, J=9 -> 2e-2). Far blocks use the
# constant-color-factor approximation (1 + mean Cc) * Kg, shipped direct.
NEAR = list(range(0, 9)) + list(range(12, 34))
NFAR = [i for i in range(NBLK) if i not in NEAR]
ITERS = 5

_CACHED_NC = None


def _build_module():
    import concourse.bass as bass
    import concourse.bacc as bacc
    import concourse.tile as tile
    from concourse import mybir
    from concourse.masks import make_identity

    f32 = mybir.dt.float32
    f16 = mybir.dt.float16
    u32 = mybir.dt.uint32
    EXP = mybir.ActivationFunctionType.Exp
    COPY = mybir.ActivationFunctionType.Copy
    ADD = mybir.AluOpType.add
    MULT = mybir.AluOpType.mult

    nc = bacc.Bacc("TRN2", target_bir_lowering=False, debug=False,
                   num_devices=NCORES)

    g_dram = nc.dram_tensor("g_feats", [C, len(NEAR) * BLK], f16,
                            kind="ExternalInput")
    h_dram = nc.dram_tensor("h_feats", [C, NLOC], f16, kind="ExternalInput")
    k16_dram = nc.dram_tensor("k16_init", [BLK, NBLK * NLOC], f16,
                              kind="ExternalInput")
    ipp_dram = nc.dram_tensor("inp_pp", [BLK, GBLK * C], f32, kind="ExternalInput")
    icn_dram = nc.dram_tensor("inp_cn", [C, NMID], f32, kind="ExternalInput")
    off_dram = nc.dram_tensor("offsets", [1, 3], u32, kind="ExternalInput")
    out_dram = nc.dram_tensor("out_loc", [BLK, (NMID // BLK) * C], f32,
                              kind="ExternalOutput")

    def bcast_inner(ap, n):
        return bass.AP(tensor=ap.tensor, offset=ap.offset, ap=[*ap.ap, [0, n]])

    with tile.TileContext(nc) as tc:
        with tc.tile_pool(name="singles", bufs=1) as singles, \
             tc.tile_pool(name="warmps", bufs=1, space="PSUM") as warmpool, \
             tc.tile_pool(name="dram", bufs=1, space="DRAM") as dram:

            # ---- long-lived SBUF state ----
            k16 = singles.tile([BLK, NBLK, NLOC], f16, name="k16")
            flat_pad = singles.tile([BLK, FPW], f16, name="flat_pad")
            g_sb = singles.tile([C, len(NEAR) * BLK], f16, name="g_sb")
            h_sb = singles.tile([C, NLOC], f16, name="h_sb")
            ipp_sb = singles.tile([BLK, GBLK * C], f32, name="ipp_sb")
            icn_sb = singles.tile([C, NMID], f32, name="icn_sb")
            ident = singles.tile([BLK, BLK], f32, name="ident")
            off_sb = singles.tile([1, 3], u32, name="off_sb")
            warm_ps = warmpool.tile([1, 512], f32, name="warm_ps")

            nc.sync.dma_start(out=ipp_sb, in_=ipp_dram[:, :])
            nc.sync.dma_start(out=icn_sb, in_=icn_dram[:, :])
            nc.sync.dma_start(out=off_sb, in_=off_dram[:, :])
            nc.sync.dma_start(out=g_sb, in_=g_dram[:, :])
            nc.sync.dma_start(out=h_sb, in_=h_dram[:, :])
            # k16 initial values (spatial gaussian factors): per near block so
            # the build pipeline starts as soon as each block lands; far
            # blocks (2 contiguous runs) need no device work at all
            for i in NEAR:
                nc.sync.dma_start(
                    out=k16[:, i, :],
                    in_=k16_dram[:, i * NLOC:(i + 1) * NLOC])
            nc.sync.dma_start(
                out=k16[:, NFAR[0]:NFAR[2] + 1, :],
                in_=k16_dram[:, NFAR[0] * NLOC:(NFAR[2] + 1) * NLOC])
            nc.sync.dma_start(
                out=k16[:, NFAR[3]:NFAR[5] + 1, :],
                in_=k16_dram[:, NFAR[3] * NLOC:(NFAR[5] + 1) * NLOC])
            make_identity(nc, ident)
            nc.vector.memset(flat_pad, 0.0)

            # runtime flat_pad element offsets: own / left / right windows
            offs = []
            for j, mx in enumerate(((PADBLK + 9 * (NCORES - 1)) * C,
                                    (PADBLK + 9 * (NCORES - 1) - HB) * C,
                                    (PADBLK + 9 * (NCORES - 1) + 9) * C)):
                regs = nc.alloc_registers(f"off_regs{j}",
                                          engines=(mybir.EngineType.DVE,))
                nc.regs_load(regs, off_sb[0:1, j:j + 1])
                offs.append(nc.snap(regs, donate=True, min_val=0, max_val=mx))
            own_off, left_off, right_off = offs

            # HAM warm-keeper: fp16 matmuls (~213 ns each) that fill PE-idle
            # windows so the activity monitor keeps the PE clock at 2.4 GHz.
            # Each warm's lhsT reads an anchor tile produced just before the
            # idle window -- without the data dependency the static scheduler
            # hoists dep-free matmuls to the very start of the Tensor queue.
            def warm(n, anchor):
                for _ in range(n):
                    nc.tensor.matmul(warm_ps, anchor[:, 0:1],
                                     k16[:, 0, 0:512], start=True, stop=True)

            def warm32(n, anchor):
                # fp32 anchor with few partitions: 128 fp32 cols = 512 cycles
                p = anchor.partition_size()
                for _ in range(n):
                    nc.tensor.matmul(warm_ps[0:1, 0:BLK], anchor[:, 0:1],
                                     ident[0:p, 0:BLK], start=True, stop=True)

            ag_in = dram.tile([BLK, (NMID // BLK) * C], f16, name="ag_in")
            ag_out = dram.tile([BLK * NCORES, (NMID // BLK) * C], f16,
                               name="ag_out")
            cc_wu_in = dram.tile([BLK, 1], f16, name="cc_wu_in")
            cc_wu_out = dram.tile([BLK * NCORES, 1], f16, name="cc_wu_out")
            # warm up the collective rings during the build phase; the first
            # AllGather otherwise pays ~35us of one-time setup on the
            # critical path
            nc.gpsimd.collective_compute(
                "AllGather",
                mybir.AluOpType.bypass,
                replica_groups=[list(range(NCORES))],
                ins=[cc_wu_in.opt()],
                outs=[cc_wu_out.opt()],
            )

            # ---- helpers ----
            def softmax_pp(pool, u_pp, mb, tag, out_dt=f32):
                """u_pp: [128, mb*C] logits, pixel-partition layout -> probs."""
                v = u_pp.rearrange("p (a c) -> p a c", c=C)
                mx = pool.tile([BLK, mb], f32, tag=f"{tag}_mx")
                nc.vector.tensor_reduce(out=mx, in_=v,
                                        axis=mybir.AxisListType.X,
                                        op=mybir.AluOpType.max)
                e = pool.tile([BLK, mb * C], f32, tag=f"{tag}_e")
                ev = e.rearrange("p (a c) -> p a c", c=C)
                nc.vector.tensor_sub(ev, v, bcast_inner(mx, C))
                nc.scalar.activation(out=e, in_=e, func=EXP)
                s = pool.tile([BLK, mb], f32, tag=f"{tag}_s")
                nc.vector.tensor_reduce(out=s, in_=ev,
                                        axis=mybir.AxisListType.X,
                                        op=mybir.AluOpType.add)
                nc.vector.reciprocal(out=s, in_=s)
                fl = pool.tile([BLK, mb * C], out_dt, tag=f"{tag}_fl")
                nc.vector.tensor_mul(fl.rearrange("p (a c) -> p a c", c=C), ev,
                                     bcast_inner(s, C))
                return fl

            # ---- phase 2: initial flat = softmax(input), all cores alike ----
            with tc.tile_pool(name="init", bufs=1) as ipool:
                fl0 = softmax_pp(ipool, ipp_sb, GBLK, "sm0", out_dt=f16)
                nc.vector.tensor_copy(
                    out=flat_pad[:, PADBLK * C:(PADBLK + GBLK) * C], in_=fl0)

            # ---- phase 1: build the fp16 K band ----
            # near block i: ccarg = G_i^T H (5-deep fp16 matmul) ->
            # t = exp(ccarg) -> k16[i] = (t + 1) * k16[i]  (init = Kg3)
            with tc.tile_pool(name="kgstage", bufs=3) as kgpool, \
                 tc.tile_pool(name="bpsum", bufs=2, space="PSUM") as bppool:
                for j, i in enumerate(NEAR):
                    gt = g_sb[:, j * BLK:(j + 1) * BLK]
                    pb = bppool.tile([BLK, 3, 512], f32, tag="pb")
                    for nb in range(3):
                        hs = h_sb[:, nb * BCH:(nb + 1) * BCH]
                        nc.tensor.matmul(pb[:, nb, 0:BCH], gt, hs,
                                         start=True, stop=True)
                    texp = kgpool.tile([BLK, NLOC], f16, tag="texp")
                    nc.scalar.activation(
                        out=texp.rearrange("p (a c) -> p a c", c=BCH),
                        in_=pb[:, :, 0:BCH], func=EXP)
                    ksl = k16[:, i, :]
                    nc.vector.scalar_tensor_tensor(
                        out=ksl, in0=texp, scalar=1.0, in1=ksl,
                        op0=ADD, op1=MULT)

            # second ring warm-up doubling as a cross-core barrier: absorbs
            # build-phase skew while iteration 0's matvec runs
            nc.gpsimd.collective_compute(
                "AllGather",
                mybir.AluOpType.bypass,
                replica_groups=[list(range(NCORES))],
                ins=[cc_wu_in.opt()],
                outs=[cc_wu_out.opt()],
            )

            # ---- phase 3: iterations ----
            with tc.tile_pool(name="iter", bufs=1) as wpool, \
                 tc.tile_pool(name="band", bufs=2) as bpool, \
                 tc.tile_pool(name="smx", bufs=2) as spool, \
                 tc.tile_pool(name="ipsum", bufs=2, space="PSUM") as ippool:
                pv_next = None
                for it in range(ITERS):
                    # matvec: comb[c, n] = sum_m K[m, n] * flat[c, m]
                    # for it > 0 the own 9 blocks were accumulated into
                    # pv_next while the AllGather was in flight
                    band = bpool.tile([BLK, NBLK * C], f16, tag="band")
                    if it == 0:
                        pv = ippool.tile([C, 3, 512], f32, tag="pv")
                        nc.vector.tensor_copy(
                            out=band[:, 0:9 * C],
                            in_=flat_pad[:, bass.ds(own_off, 9 * C)])
                        i_lo = 0
                    else:
                        pv = pv_next
                        i_lo = 9
                        nc.sync.dma_start(
                            out=flat_pad[:, PADBLK * C:(PADBLK + GBLK) * C]
                            .rearrange("p (r j) -> p r j", r=NCORES),
                            in_=ag_out.rearrange("(r p) j -> p r j", p=BLK))
                    nc.vector.tensor_copy(
                        out=band[:, 9 * C:(9 + HB) * C],
                        in_=flat_pad[:, bass.ds(left_off, HB * C)])
                    nc.vector.tensor_copy(
                        out=band[:, (9 + HB) * C:NBLK * C],
                        in_=flat_pad[:, bass.ds(right_off, HB * C)])
                    for nb in range(3):
                        for i in range(i_lo, NBLK):
                            nc.tensor.matmul(
                                pv[:, nb, 0:CHS[nb]],
                                band[:, i * C:(i + 1) * C],
                                k16[:, i, CH0[nb]:CH0[nb] + CHS[nb]],
                                start=(it == 0 and i == 0),
                                stop=(i == NBLK - 1))

                    # 3x3 box sum, x-pass per row-aligned PSUM chunk
                    # (hardware: at most one PSUM operand per vector op, so
                    # seed t1 with a scalar-engine copy then accumulate);
                    # y-pass and transposes are chunk-pipelined so only
                    # chunk 2's tail is serial after the matvec
                    t1 = wpool.tile([C, NLOC], f32, tag="t1")
                    t1r = t1.rearrange("p (row x) -> p row x", x=W)
                    u = wpool.tile([C, NMID], f32, tag="u")
                    pt = ippool.tile([BLK, (NMID // BLK) * C], f32, tag="pt",
                                     bufs=1)
                    r0 = 0
                    for nb in range(3):
                        nr = CHROWS[nb]
                        cb = pv[:, nb, 0:CHS[nb]].rearrange(
                            "p (row x) -> p row x", x=W)
                        tb = t1r[:, r0:r0 + nr, :]
                        nc.scalar.activation(out=tb, in_=cb, func=COPY)
                        nc.vector.tensor_add(tb[:, :, 1:W - 1],
                                             tb[:, :, 1:W - 1],
                                             cb[:, :, 0:W - 2])
                        nc.vector.tensor_add(tb[:, :, 1:W - 1],
                                             tb[:, :, 1:W - 1],
                                             cb[:, :, 2:W])
                        nc.vector.tensor_add(tb[:, :, 0:1], tb[:, :, 0:1],
                                             cb[:, :, 0:1])
                        nc.vector.tensor_add(tb[:, :, 0:1], tb[:, :, 0:1],
                                             cb[:, :, 1:2])
                        nc.vector.tensor_add(tb[:, :, W - 1:W],
                                             tb[:, :, W - 1:W],
                                             cb[:, :, W - 1:W])
                        nc.vector.tensor_add(tb[:, :, W - 1:W],
                                             tb[:, :, W - 1:W],
                                             cb[:, :, W - 2:W - 1])
                        r0 += nr
                        if nb == 0:
                            continue
                        if nb == 1:
                            # u rows 0-7 (needs t1 rows 0-9) + transposes 0-5
                            a, b = 0, 8 * W
                        else:
                            # u rows 8-11 (needs t1 rows 8-13) + transposes 6-8
                            a, b = 8 * W, NMID
                        nc.vector.tensor_add(u[:, a:b], t1[:, a:b],
                                             t1[:, a + 2 * W:b + 2 * W])
                        nc.vector.tensor_add(u[:, a:b], u[:, a:b],
                                             t1[:, a + W:b + W])
                        nc.vector.tensor_add(u[:, a:b], u[:, a:b],
                                             icn_sb[:, a:b])
                        for kb in range(a // BLK, b // BLK):
                            nc.tensor.transpose(pt[:, kb * C:(kb + 1) * C],
                                                u[:, kb * BLK:(kb + 1) * BLK],
                                                ident[0:C, 0:C])
                        warm32(4, u)

                    if it < ITERS - 1:
                        flat_l = softmax_pp(spool, pt, NMID // BLK, "smx",
                                            out_dt=f16)
                        nc.sync.dma_start(out=ag_in, in_=flat_l)
                        nc.gpsimd.collective_compute(
                            "AllGather",
                            mybir.AluOpType.bypass,
                            replica_groups=[list(range(NCORES))],
                            ins=[ag_in.opt()],
                            outs=[ag_out.opt()],
                        )
                        # overlap the AllGather with next iteration's own-
                        # block matmul accumulation (flat_l is local data)
                        pv_next = ippool.tile([C, 3, 512], f32, tag="pv")
                        for nb in range(3):
                            for j in range(9):
                                nc.tensor.matmul(
                                    pv_next[:, nb, 0:CHS[nb]],
                                    flat_l[:, j * C:(j + 1) * C],
                                    k16[:, j, CH0[nb]:CH0[nb] + CHS[nb]],
                                    start=(j == 0), stop=False)
                        warm(18, flat_l)
                    else:
                        flat_l = softmax_pp(spool, pt, NMID // BLK, "smx",
                                            out_dt=f32)
                        nc.sync.dma_start(out=out_dram[:, :], in_=flat_l)

    nc.compile()
    return nc


def _host_inputs(input_tensor, reference_tensor):
    logits = np.ascontiguousarray(
        np.asarray(input_tensor, dtype=np.float32)[0].reshape(C, N))
    ref = np.asarray(reference_tensor, dtype=np.float32)[0]  # [3, 96, 96]

    RGB = (ref / 0.5).reshape(3, N).astype(np.float32)
    c2 = (-0.5 * (RGB * RGB).sum(axis=0)).astype(np.float32)
    ones = np.ones(N, np.float32)
    G_all = np.stack([RGB[0], RGB[1], RGB[2], c2, ones]).astype(np.float16)
    H_all = np.stack([RGB[0], RGB[1], RGB[2], ones, c2]).astype(np.float16)

    # mean color-kernel value for the far-block constant approximation
    samp = RGB[:, ::37]
    d2 = ((samp[:, :, None] - samp[:, None, :]) ** 2).sum(axis=0)
    tbar = float(np.exp(-0.5 * d2).mean())

    # input in pixel-partition layout [128, 72*5]
    ipp = np.ascontiguousarray(
        logits.reshape(C, GBLK, BLK).transpose(2, 1, 0).reshape(BLK, GBLK * C))

    # spatial gaussian tables; x table carries the 3.0 UPDATE_FACTOR fold
    dtab = np.exp(-(np.arange(-(H - 1), H) ** 2) / 50.0)
    gy1 = dtab.astype(np.float32)
    gx3 = (3.0 * dtab).astype(np.float32)
    yy_all = (np.arange(N) // W).astype(np.int64)
    xx_all = (np.arange(N) % W).astype(np.int64)

    def k16_for_core(r, order, yext):
        k = np.zeros((BLK, NBLK, NLOC), np.float16)
        xn = np.arange(W)
        for i, gb in enumerate(order):
            if 0 <= gb < GBLK:
                pm = np.arange(gb * BLK, (gb + 1) * BLK)
                A = gy1[yy_all[pm][:, None] - yext[None, :] + H - 1]
                B = gx3[xx_all[pm][:, None] - xn[None, :] + H - 1]
                kg = (A[:, :, None] * B[:, None, :]).reshape(BLK, NLOC)
                if i in NFAR:
                    kg *= (1.0 + tbar)
                k[:, i, :] = kg.astype(np.float16)
        return k.reshape(BLK, NBLK * NLOC)

    in_maps = []
    k16_interior = None
    for r in range(NCORES):
        # band-local order: [own 9 | left 14 | right 14] global blocks
        order = (list(range(9 * r, 9 * r + 9))
                 + list(range(9 * r - HB, 9 * r))
                 + list(range(9 * r + 9, 9 * r + 9 + HB)))
        yext = np.clip(np.arange(RPC * r - 1, RPC * (r + 1) + 1), 0, H - 1)
        g = np.zeros((C, len(NEAR) * BLK), np.float16)
        for j, i in enumerate(NEAR):
            gb = order[i]
            if 0 <= gb < GBLK:
                g[:, j * BLK:(j + 1) * BLK] = G_all[:, gb * BLK:(gb + 1) * BLK]
        if 2 <= r <= 5:
            if k16_interior is None:
                k16_interior = k16_for_core(r, order, yext)
            k16 = k16_interior
        else:
            k16 = k16_for_core(r, order, yext)
        hpix = (yext[:, None] * W + np.arange(W)[None, :]).reshape(-1)
        h = np.ascontiguousarray(H_all[:, hpix])
        icn = np.ascontiguousarray(
            logits.reshape(C, H, W)[:, RPC * r:RPC * (r + 1), :].reshape(C, NMID))
        offsets = np.array([[(PADBLK + 9 * r) * C,
                             9 * r * C,
                             (PADBLK + 9 * r + 9) * C]], np.uint32)
        in_maps.append({
            "g_feats": g,
            "h_feats": h,
            "k16_init": k16,
            "inp_pp": ipp,
            "inp_cn": icn,
            "offsets": offsets,
        })
    return in_maps


def _assemble(results):
    out = np.empty((C, N), np.float32)
    for r in range(NCORES):
        blk = results[r]["out_loc"].reshape(BLK, NMID // BLK, C)
        out[:, NMID * r:NMID * (r + 1)] = (
            blk.transpose(2, 1, 0).reshape(C, NMID))
    return out.reshape(1, C, H, W)


def _get_nc():
    global _CACHED_NC
    if _CACHED_NC is None:
        _CACHED_NC = _build_module()
    return _CACHED_NC


def run(input_tensor, reference_tensor, trace=False):
    from concourse.bass_utils import run_bass_kernel_spmd
    nc = _get_nc()
    in_maps = _host_inputs(input_tensor, reference_tensor)
    res = run_bass_kernel_spmd(nc, in_maps, core_ids=list(range(NCORES)),
                               trace=trace)
    return _assemble(res.results), res


def kernel(input_tensor, reference_tensor):
    out, _ = run(input_tensor, reference_tensor, trace=False)
    return out
